# revision 20
# baseline (speedup 1.0000x reference)
"""Trainium2 Bass kernel for nn_ETypePromptModel: logits = einsum('bpd,cpd->bc').

Equivalent to X @ W.T with X=[B, K]=[16384, 256], W=[C, K]=[4096, 256],
K = L*D = 256. Data-parallel over B across 8 NeuronCores; W replicated.

bf16 plan (rel-err gate is 2e-2; bf16 end-to-end lands ~3e-3):
  - Host casts X/W to bf16, lays them out K-major, and packs them into
    four DMA-shaped tensors so each DMA ring carries exactly ONE
    startup-critical transfer (per-DMA ring overhead is ~1.5 us, so
    chunk count matters more than chunk size):
      critA [128, 2560] = X^T k0 cols 0:512 | W^T k0 cols 0:2048 (sync)
      critB [128, 2560] = X^T k1 cols 0:512 | W^T k1 cols 0:2048 (scalar)
      xrest [128, 3072] = X^T k0 cols 512:2048 | k1  (sync, behind critA)
      wh1   [128, 4096] = W^T both k cols 2048:4096  (scalar, behind critB)
    The gpsimd (SWDGE) ring arms too late (~11.8 us) for inputs and is
    reserved for the output stream. Output is written bf16 and upcast
    to fp32 on the host. Per-core DRAM traffic: 3.15 MB in + 16.8 MB
    out vs 39.5 MB for fp32.
  - PE: 256 bf16 matmuls ([128k x 128b] stationary, [128k x 512c]
    moving, fp32 PSUM) at the 216 ns/MM streaming rate = 55.3 us warm
    @ 2.4 GHz -- the bf16 ridge (78.6 TF/s / 358 GB/s ~ 219 flop/B vs
    217 here), PE and HBM simultaneously near-saturated.
  - c-dimension in two half-column passes (jh-outer) so the startup
    needs only the W h0 halves; m-tiles 0-3 x (jj0,jj1) run k-major
    across all 8 PSUM banks (bridge) so the PE streams k0 work (critA)
    while critB is still in flight on the later-arming scalar ring;
    their (jj2,jj3) columns follow as a second mini-pass.
    Measured: first real MM ~12.2 us, stream 55.2 us (floor), HAM warm
    throughout, ~9 us fixed exit-barrier after the last DMA; 73.7 us
    total vs the 112.4 us fp32 baseline.
  - Elsewhere k0/k1 run back-to-back per bank, freeing banks evenly so
    the PSUM->SBUF copies (Vector/Scalar alternating, cast to bf16)
    never gate the next m-tile. Scalar issues no mid-stream DMAs
    (head-of-line blocking); output rows go out as 0.5 MB half-row
    DMAs alternating sync/gpsimd, and the final tile's two quarter
    writes ride scalar's own empty ring right behind its copies.
  - Junk warmup matmuls keep the PE busy from the preamble end to the
    stream start so the HAM clock gate is 8/8 from the first real MM.
"""

import sys

import numpy as np

sys.path.insert(0, "/opt/trn_rl_repo")

B, C, L, D = 16384, 4096, 2, 128
K = L * D  # 256 contraction length
N_CORES = 8
B_LOC = B // N_CORES  # 2048
P = 128
KT = K // P  # 2 k-tiles
M_TILES = B_LOC // P  # 16
N_TILE = 512  # moving free dim per matmul (PSUM bank = 512 fp32)
JH_TILES = 4  # c-tiles per half-column pass
CH = C // 2  # 2048 (half-columns)
XP = 512  # X first-cols chunk (stationaries for m-tiles 0-3)
XR = B_LOC - XP  # 1536
WARMUP_MMS = 8

_CACHE = {}
PROFILE = False
TRACE_ALL_CORES = False
LAST_RESULT = None


def _build():
    import concourse.mybir as mybir
    import concourse.tile as tile
    from concourse import bacc

    f32 = mybir.dt.float32
    bf16 = mybir.dt.bfloat16

    nc = bacc.Bacc(
        "TRN2",
        target_bir_lowering=False,
        debug=False,
        enable_asserts=False,
        num_devices=N_CORES,
    )

    critA_d = nc.dram_tensor("critA", [P, XP + CH], bf16, kind="ExternalInput").ap()
    critB_d = nc.dram_tensor("critB", [P, XP + CH], bf16, kind="ExternalInput").ap()
    xrest_d = nc.dram_tensor("xrest", [P, 2 * XR], bf16, kind="ExternalInput").ap()
    wh1_d = nc.dram_tensor("wh1", [P, 2 * CH], bf16, kind="ExternalInput").ap()
    out_dram = nc.dram_tensor("out", [B_LOC, C], bf16, kind="ExternalOutput").ap()

    with tile.TileContext(nc) as tc:
        with (
            tc.tile_pool(name="cst", bufs=1) as cst_pool,
            tc.tile_pool(name="big", bufs=1) as big_pool,
            tc.tile_pool(name="osb", bufs=12) as out_pool,
            tc.tile_pool(name="psm", bufs=8, space="PSUM") as psum_pool,
        ):
            # --- PE warmup: junk matmuls keep HAM at 8/8 until inputs land
            junk = cst_pool.tile([P, N_TILE], bf16, name="junk")
            nc.vector.memset(junk, 0.0)
            warm_ps = psum_pool.tile([P, N_TILE], f32, tag="pmm", name="warm_ps")
            for _ in range(WARMUP_MMS):
                nc.tensor.matmul(warm_ps, junk[:, :P], junk, start=True, stop=True)

            # --- input loads: ONE dma per ring for the startup-critical set
            critA = big_pool.tile([P, XP + CH], bf16, name="critA")
            critB = big_pool.tile([P, XP + CH], bf16, name="critB")
            xrest = big_pool.tile([P, 2 * XR], bf16, name="xrest")
            wh1 = big_pool.tile([P, 2 * CH], bf16, name="wh1")
            # critA first on sync, critB first on scalar; the later-needed
            # rest queues behind them (xrest by m-tile 4 ~18 us -> sync;
            # wh1 by the second pass ~40 us -> scalar); gpsimd's ring
            # stays clear for the output stream
            # each crit tensor loads in two pieces: the bridge's k0/k1
            # phases gate (via subtile deps) on just the first piece
            # (X first-cols + W j0/j1 chunks), pulling the stream start
            # ~1 us earlier; the j2/j3 chunks follow before the B2 pass
            SPL = XP + CH // 2  # 1536
            nc.sync.dma_start(critA[:, 0:SPL], critA_d[:, 0:SPL])
            nc.scalar.dma_start(critB[:, 0:SPL], critB_d[:, 0:SPL])
            nc.sync.dma_start(critA[:, SPL:], critA_d[:, SPL:])
            nc.scalar.dma_start(critB[:, SPL:], critB_d[:, SPL:])
            nc.sync.dma_start(xrest, xrest_d)
            nc.scalar.dma_start(wh1, wh1_d)

            def stat(mt, k):
                if mt < XP // P:
                    src = critA if k == 0 else critB
                    return src[:, mt * P : (mt + 1) * P]
                m = mt - XP // P
                return xrest[:, k * XR + m * P : k * XR + (m + 1) * P]

            def wsl(jh, k, jj):
                lo = jj * N_TILE
                if jh == 0:
                    src = critA if k == 0 else critB
                    return src[:, XP + lo : XP + lo + N_TILE]
                return wh1[:, k * CH + lo : k * CH + lo + N_TILE]

            # --- main stream: two half-column passes over the m-tiles
            def evac(jh, mt, pms, jjs=range(JH_TILES), last=False):
                """PSUM -> SBUF copies (Vector/Scalar) + output DMA."""
                n = len(list(jjs))
                out_sb = out_pool.tile(
                    [P, n * N_TILE], bf16, tag="osb", name="out_sb"
                )
                base = jh * CH + list(jjs)[0] * N_TILE
                row = out_dram[mt * P : (mt + 1) * P, base : base + n * N_TILE]
                for i, jj in enumerate(jjs):
                    sl = slice(i * N_TILE, (i + 1) * N_TILE)
                    if i % 2 == 0:
                        nc.vector.tensor_copy(out=out_sb[:, sl], in_=pms[jj])
                    else:
                        nc.scalar.copy(out_sb[:, sl], pms[jj])
                    if last and i == 1:
                        # final write, first half: sync's ring is idle by
                        # now, fire right after the first two copies
                        h = slice(0, 2 * N_TILE)
                        nc.sync.dma_start(row[:, h], out_sb[:, h])
                if last:
                    # second half on scalar's own ring behind its last copy
                    h = slice(2 * N_TILE, n * N_TILE)
                    nc.scalar.dma_start(row[:, h], out_sb[:, h])
                else:
                    # scalar stays DMA-free mid-stream: a dma_start waiting
                    # on the vector-side copies would head-of-line block
                    # the next m-tile's scalar copies
                    ring = nc.sync if mt % 2 == 0 else nc.gpsimd
                    ring.dma_start(row, out_sb)

            def mk_pms(n=JH_TILES):
                return [
                    psum_pool.tile([P, N_TILE], f32, tag="pmm", name="pmm")
                    for _ in range(n)
                ]

            for jh in range(2):
                if jh == 0:
                    # bridge B1: m-tiles 0-3 x (jj0,jj1) k-major across all
                    # 8 banks -- 3.46 us of k0 (critA) streaming covers
                    # critB's in-flight time on the later-arming scalar ring
                    pmss = [mk_pms(2) for _ in range(4)]
                    for k in range(KT):
                        for m in range(4):
                            for jj in range(2):
                                nc.tensor.matmul(
                                    pmss[m][jj],
                                    stat(m, k),
                                    wsl(0, k, jj),
                                    start=(k == 0),
                                    stop=(k == KT - 1),
                                )
                    for m in range(4):
                        evac(0, m, {0: pmss[m][0], 1: pmss[m][1]}, jjs=(0, 1))
                    # B2: the deferred (jj2,jj3) columns of m-tiles 0-3
                    for m in range(4):
                        pms = {}
                        for jj in (2, 3):
                            pms[jj] = psum_pool.tile(
                                [P, N_TILE], f32, tag="pmm", name="pmm"
                            )
                            for k in range(KT):
                                nc.tensor.matmul(
                                    pms[jj],
                                    stat(m, k),
                                    wsl(0, k, jj),
                                    start=(k == 0),
                                    stop=(k == KT - 1),
                                )
                        evac(0, m, pms, jjs=(2, 3))
                    mts = range(4, M_TILES)
                else:
                    mts = range(M_TILES)
                for mt in mts:
                    pms = mk_pms()
                    for jj in range(JH_TILES):
                        for k in range(KT):
                            nc.tensor.matmul(
                                pms[jj],
                                stat(mt, k),
                                wsl(jh, k, jj),
                                start=(k == 0),
                                stop=(k == KT - 1),
                            )
                    evac(jh, mt, pms, last=(jh == 1 and mt == M_TILES - 1))

    nc.compile()
    return nc


def kernel(batchs, label2embed):
    global LAST_RESULT
    import ml_dtypes

    from concourse.bass_utils import run_bass_kernel_spmd

    bf16 = ml_dtypes.bfloat16

    if "nc" not in _CACHE:
        _CACHE["nc"] = _build()
    nc = _CACHE["nc"]

    X = np.ascontiguousarray(batchs, dtype=np.float32).reshape(B, K)
    W = np.ascontiguousarray(label2embed, dtype=np.float32).reshape(C, K)
    assert X.shape == (B, K) and W.shape == (C, K)

    wt = W.astype(bf16).T  # [K, C] view
    wh1 = np.ascontiguousarray(
        np.concatenate([wt[0:P, CH:C], wt[P : 2 * P, CH:C]], axis=1)
    )
    wk0h0 = wt[0:P, 0:CH]
    wk1h0 = wt[P : 2 * P, 0:CH]
    Xb = X.astype(bf16)

    in_maps = []
    for c in range(N_CORES):
        xtc = Xb[c * B_LOC : (c + 1) * B_LOC].T  # [K, B_LOC] view
        critA = np.ascontiguousarray(
            np.concatenate([xtc[0:P, 0:XP], wk0h0], axis=1)
        )
        critB = np.ascontiguousarray(
            np.concatenate([xtc[P : 2 * P, 0:XP], wk1h0], axis=1)
        )
        xrest = np.ascontiguousarray(
            np.concatenate([xtc[0:P, XP:B_LOC], xtc[P : 2 * P, XP:B_LOC]], axis=1)
        )
        in_maps.append({"critA": critA, "critB": critB, "xrest": xrest, "wh1": wh1})
    res = run_bass_kernel_spmd(
        nc,
        in_maps,
        core_ids=list(range(N_CORES)),
        trace=PROFILE,
        trace_cores=list(range(N_CORES)) if (PROFILE and TRACE_ALL_CORES) else None,
    )
    LAST_RESULT = res
    out = np.concatenate([r["out"] for r in res.results], axis=0)
    return out.astype(np.float32)


# revision 21
# speedup vs baseline: 1.0101x; 1.0101x over previous
"""Trainium2 Bass kernel for nn_ETypePromptModel: logits = einsum('bpd,cpd->bc').

Equivalent to X @ W.T with X=[B, K]=[16384, 256], W=[C, K]=[4096, 256],
K = L*D = 256. Data-parallel over B across 8 NeuronCores; W replicated.

bf16 plan (rel-err gate is 2e-2; bf16 end-to-end lands ~3e-3):
  - Host casts X/W to bf16, lays them out K-major, and packs them into
    four DMA-shaped tensors so each DMA ring carries exactly ONE
    startup-critical transfer (per-DMA ring overhead is ~1.5 us, so
    chunk count matters more than chunk size):
      critA [128, 2560] = X^T k0 cols 0:512 | W^T k0 cols 0:2048 (sync)
      critB [128, 2560] = X^T k1 cols 0:512 | W^T k1 cols 0:2048 (scalar)
      xrest [128, 3072] = X^T k0 cols 512:2048 | k1  (sync, behind critA)
      wh1   [128, 4096] = W^T both k cols 2048:4096  (scalar, behind critB)
    The gpsimd (SWDGE) ring arms too late (~11.8 us) for inputs and is
    reserved for the output stream. Output is written bf16 and upcast
    to fp32 on the host. Per-core DRAM traffic: 3.15 MB in + 16.8 MB
    out vs 39.5 MB for fp32.
  - PE: 256 bf16 matmuls ([128k x 128b] stationary, [128k x 512c]
    moving, fp32 PSUM) at the 216 ns/MM streaming rate = 55.3 us warm
    @ 2.4 GHz -- the bf16 ridge (78.6 TF/s / 358 GB/s ~ 219 flop/B vs
    217 here), PE and HBM simultaneously near-saturated.
  - c-dimension in two half-column passes (jh-outer) so the startup
    needs only the W h0 halves; m-tiles 0-3 x (jj0,jj1) run k-major
    across all 8 PSUM banks (bridge) so the PE streams k0 work (critA)
    while critB is still in flight on the later-arming scalar ring;
    their (jj2,jj3) columns follow as a second mini-pass.
    Measured: first real MM ~12.2 us, stream 55.2 us (floor), HAM warm
    throughout, ~9 us fixed exit-barrier after the last DMA; 73.7 us
    total vs the 112.4 us fp32 baseline.
  - Elsewhere k0/k1 run back-to-back per bank, freeing banks evenly so
    the PSUM->SBUF copies (Vector/Scalar alternating, cast to bf16)
    never gate the next m-tile. Scalar issues no mid-stream DMAs
    (head-of-line blocking); output rows go out as 0.5 MB half-row
    DMAs alternating sync/gpsimd, and the final tile's two quarter
    writes ride scalar's own empty ring right behind its copies.
  - Junk warmup matmuls keep the PE busy from the preamble end to the
    stream start so the HAM clock gate is 8/8 from the first real MM.
"""

import sys

import numpy as np

sys.path.insert(0, "/opt/trn_rl_repo")

B, C, L, D = 16384, 4096, 2, 128
K = L * D  # 256 contraction length
N_CORES = 8
B_LOC = B // N_CORES  # 2048
P = 128
KT = K // P  # 2 k-tiles
M_TILES = B_LOC // P  # 16
N_TILE = 512  # moving free dim per matmul (PSUM bank = 512 fp32)
JH_TILES = 4  # c-tiles per half-column pass
CH = C // 2  # 2048 (half-columns)
XP = 512  # X first-cols chunk (stationaries for m-tiles 0-3)
XR = B_LOC - XP  # 1536
WARMUP_MMS = 8

_CACHE = {}
PROFILE = False
TRACE_ALL_CORES = False
LAST_RESULT = None


def _build():
    import concourse.mybir as mybir
    import concourse.tile as tile
    from concourse import bacc

    f32 = mybir.dt.float32
    bf16 = mybir.dt.bfloat16

    nc = bacc.Bacc(
        "TRN2",
        target_bir_lowering=False,
        debug=False,
        enable_asserts=False,
        num_devices=N_CORES,
    )

    critA_d = nc.dram_tensor("critA", [P, XP + CH], bf16, kind="ExternalInput").ap()
    critB_d = nc.dram_tensor("critB", [P, XP + CH], bf16, kind="ExternalInput").ap()
    xrest_d = nc.dram_tensor("xrest", [P, 2 * XR], bf16, kind="ExternalInput").ap()
    wh1_d = nc.dram_tensor("wh1", [P, 2 * CH], bf16, kind="ExternalInput").ap()
    out_dram = nc.dram_tensor("out", [B_LOC, C], bf16, kind="ExternalOutput").ap()

    with tile.TileContext(nc) as tc:
        with (
            tc.tile_pool(name="cst", bufs=1) as cst_pool,
            tc.tile_pool(name="big", bufs=1) as big_pool,
            tc.tile_pool(name="osb", bufs=12) as out_pool,
            tc.tile_pool(name="psm", bufs=8, space="PSUM") as psum_pool,
        ):
            # --- PE warmup: junk matmuls keep HAM at 8/8 until inputs land
            junk = cst_pool.tile([P, N_TILE], bf16, name="junk")
            nc.vector.memset(junk, 0.0)
            warm_ps = psum_pool.tile([P, N_TILE], f32, tag="pmm", name="warm_ps")
            for _ in range(WARMUP_MMS):
                nc.tensor.matmul(warm_ps, junk[:, :P], junk, start=True, stop=True)

            # --- input loads: ONE dma per ring for the startup-critical set
            critA = big_pool.tile([P, XP + CH], bf16, name="critA")
            critB = big_pool.tile([P, XP + CH], bf16, name="critB")
            xrest = big_pool.tile([P, 2 * XR], bf16, name="xrest")
            wh1 = big_pool.tile([P, 2 * CH], bf16, name="wh1")
            # critA first on sync, critB first on scalar; the later-needed
            # rest queues behind them (xrest by m-tile 4 ~18 us -> sync;
            # wh1 by the second pass ~40 us -> scalar); gpsimd's ring
            # stays clear for the output stream
            nc.sync.dma_start(critA, critA_d)
            nc.scalar.dma_start(critB, critB_d)
            nc.sync.dma_start(xrest, xrest_d)
            nc.scalar.dma_start(wh1, wh1_d)

            def stat(mt, k):
                if mt < XP // P:
                    src = critA if k == 0 else critB
                    return src[:, mt * P : (mt + 1) * P]
                m = mt - XP // P
                return xrest[:, k * XR + m * P : k * XR + (m + 1) * P]

            def wsl(jh, k, jj):
                lo = jj * N_TILE
                if jh == 0:
                    src = critA if k == 0 else critB
                    return src[:, XP + lo : XP + lo + N_TILE]
                return wh1[:, k * CH + lo : k * CH + lo + N_TILE]

            # --- main stream: two half-column passes over the m-tiles
            def evac(jh, mt, pms, jjs=range(JH_TILES), last=False):
                """PSUM -> SBUF copies (Vector/Scalar) + output DMA."""
                n = len(list(jjs))
                out_sb = out_pool.tile(
                    [P, n * N_TILE], bf16, tag="osb", name="out_sb"
                )
                base = jh * CH + list(jjs)[0] * N_TILE
                row = out_dram[mt * P : (mt + 1) * P, base : base + n * N_TILE]
                for i, jj in enumerate(jjs):
                    sl = slice(i * N_TILE, (i + 1) * N_TILE)
                    if i % 2 == 0:
                        nc.vector.tensor_copy(out=out_sb[:, sl], in_=pms[jj])
                    else:
                        nc.scalar.copy(out_sb[:, sl], pms[jj])
                    if last and i == 1:
                        # final write, first half: sync's ring is idle by
                        # now, fire right after the first two copies
                        h = slice(0, 2 * N_TILE)
                        nc.sync.dma_start(row[:, h], out_sb[:, h])
                if last:
                    # second half on scalar's own ring behind its last copy
                    h = slice(2 * N_TILE, n * N_TILE)
                    nc.scalar.dma_start(row[:, h], out_sb[:, h])
                else:
                    # scalar stays DMA-free mid-stream: a dma_start waiting
                    # on the vector-side copies would head-of-line block
                    # the next m-tile's scalar copies
                    ring = nc.sync if mt % 2 == 0 else nc.gpsimd
                    ring.dma_start(row, out_sb)

            def mk_pms(n=JH_TILES):
                return [
                    psum_pool.tile([P, N_TILE], f32, tag="pmm", name="pmm")
                    for _ in range(n)
                ]

            for jh in range(2):
                if jh == 0:
                    # bridge B1: m-tiles 0-3 x (jj0,jj1) k-major across all
                    # 8 banks -- 3.46 us of k0 (critA) streaming covers
                    # critB's in-flight time on the later-arming scalar ring
                    pmss = [mk_pms(2) for _ in range(4)]
                    for k in range(KT):
                        for m in range(4):
                            for jj in range(2):
                                nc.tensor.matmul(
                                    pmss[m][jj],
                                    stat(m, k),
                                    wsl(0, k, jj),
                                    start=(k == 0),
                                    stop=(k == KT - 1),
                                )
                    for m in range(4):
                        evac(0, m, {0: pmss[m][0], 1: pmss[m][1]}, jjs=(0, 1))
                    # B2: the deferred (jj2,jj3) columns of m-tiles 0-3
                    for m in range(4):
                        pms = {}
                        for jj in (2, 3):
                            pms[jj] = psum_pool.tile(
                                [P, N_TILE], f32, tag="pmm", name="pmm"
                            )
                            for k in range(KT):
                                nc.tensor.matmul(
                                    pms[jj],
                                    stat(m, k),
                                    wsl(0, k, jj),
                                    start=(k == 0),
                                    stop=(k == KT - 1),
                                )
                        evac(0, m, pms, jjs=(2, 3))
                    mts = range(4, M_TILES)
                else:
                    mts = range(M_TILES)
                for mt in mts:
                    pms = mk_pms()
                    for jj in range(JH_TILES):
                        for k in range(KT):
                            nc.tensor.matmul(
                                pms[jj],
                                stat(mt, k),
                                wsl(jh, k, jj),
                                start=(k == 0),
                                stop=(k == KT - 1),
                            )
                    evac(jh, mt, pms, last=(jh == 1 and mt == M_TILES - 1))

    nc.compile()
    return nc


def kernel(batchs, label2embed):
    global LAST_RESULT
    import ml_dtypes

    from concourse.bass_utils import run_bass_kernel_spmd

    bf16 = ml_dtypes.bfloat16

    if "nc" not in _CACHE:
        _CACHE["nc"] = _build()
    nc = _CACHE["nc"]

    X = np.ascontiguousarray(batchs, dtype=np.float32).reshape(B, K)
    W = np.ascontiguousarray(label2embed, dtype=np.float32).reshape(C, K)
    assert X.shape == (B, K) and W.shape == (C, K)

    wt = W.astype(bf16).T  # [K, C] view
    wh1 = np.ascontiguousarray(
        np.concatenate([wt[0:P, CH:C], wt[P : 2 * P, CH:C]], axis=1)
    )
    wk0h0 = wt[0:P, 0:CH]
    wk1h0 = wt[P : 2 * P, 0:CH]
    Xb = X.astype(bf16)

    in_maps = []
    for c in range(N_CORES):
        xtc = Xb[c * B_LOC : (c + 1) * B_LOC].T  # [K, B_LOC] view
        critA = np.ascontiguousarray(
            np.concatenate([xtc[0:P, 0:XP], wk0h0], axis=1)
        )
        critB = np.ascontiguousarray(
            np.concatenate([xtc[P : 2 * P, 0:XP], wk1h0], axis=1)
        )
        xrest = np.ascontiguousarray(
            np.concatenate([xtc[0:P, XP:B_LOC], xtc[P : 2 * P, XP:B_LOC]], axis=1)
        )
        in_maps.append({"critA": critA, "critB": critB, "xrest": xrest, "wh1": wh1})
    res = run_bass_kernel_spmd(
        nc,
        in_maps,
        core_ids=list(range(N_CORES)),
        trace=PROFILE,
        trace_cores=list(range(N_CORES)) if (PROFILE and TRACE_ALL_CORES) else None,
    )
    LAST_RESULT = res
    out = np.concatenate([r["out"] for r in res.results], axis=0)
    return out.astype(np.float32)
